# revision 22
# baseline (speedup 1.0000x reference)
"""AssemblyClassifier Trainium2 kernel: 8-way batch-parallel across NeuronCores.

Per core (batch b), x = input_seq[b] : (28, 16384, 8) f32 is viewed flat as
[112 partitions = 4*e + t_hi, (t_lo, f)] (a contiguous split, so one full-rate
128-partition DMA per 512-t_lo chunk).  The assembly fold
G2 = [-8*scale*Ef | alpha*(1-Ef)].T @ eq_classes is computed on HOST (tiny
matmul) and shipped pre-masked per t_hi group as a [128, 4*512] bf16 table, so
the device preamble is one 512KB DMA.  Per chunk: the feature sum runs as one
DVE tensor_reduce over the innermost f=8 axis; GpSimd (otherwise idle)
computes the NaN mask (is_equal); DVE does the fused min/mult select to bf16;
32 K=112 matmuls per chunk use the 4 pre-masked tables with matmul columns
t-interleaved (j::4) so each out-DMA partition writes contiguous 2KB runs;
PSUM evac runs on ACT.  Read DMAs ride the
SP HWDGE ring (nc.sync) and output writes ride the ACT ring (nc.scalar), so
the two streams round-robin on the 16 SDMA engines and overlap instead of
serializing (the previous version put both on one ring: 82us DMA span for
23MB).  Output is written bf16 and upcast to f32 on host.
"""
import os
import sys
import types

import numpy as np

_B, _E, _T, _F = 8, 28, 16384, 8
_A, _C = 1024, 256
_HI = 4                 # t_hi groups (partition dim = 4*e + t_hi)
_TL = _T // _HI         # 4096 t_lo per group
_NT = 512               # t_lo chunk
_NJ = _NT // 128        # 128-col groups per chunk
_NCHUNK = _TL // _NT    # 8
_P = _E * _HI           # 112 active partitions

_cache = {}
LAST_RESULTS = None


def _ensure_axon_hooks():
    """The RL image's antenv lacks axon_hooks; shim it so trace=True works."""
    try:
        import antenv.axon_hooks  # noqa: F401
        return
    except Exception:
        pass
    try:
        from trn_agent_boot.trn_boot import _ntff_profile_via_ctypes
        hook = _ntff_profile_via_ctypes('/opt/axon/libaxon_pjrt.so')
    except Exception:
        hook = None
    m = types.ModuleType('antenv.axon_hooks')
    m.get_axon_ntff_profile_hook = lambda: hook
    m.set_axon_ntff_profile_hook = lambda h: None
    sys.modules['antenv.axon_hooks'] = m


def _build():
    import concourse.bass as bass
    import concourse.mybir as mybir
    from concourse import bacc
    from concourse.tile import TileContext

    F32 = mybir.dt.float32
    BF16 = mybir.dt.bfloat16
    ALU = mybir.AluOpType

    nc = bacc.Bacc("TRN2", target_bir_lowering=False)
    x = nc.declare_dram_parameter("x", [_E, _T, _F], F32, isOutput=False)
    g2 = nc.declare_dram_parameter("g2", [128, _HI * 512], BF16, isOutput=False)
    # transposed output: out[c, q*2048 + g*512 + tl]; host un-transposes.
    # Each chunk writes two [128c, 4KB-contiguous] DMAs - max-efficiency runs.
    out = nc.declare_dram_parameter("out", [_C, _T], BF16, isOutput=True)

    # flat [112, (t_lo f)] view of x; partition p = 4*e + t_hi
    xv = x[:].rearrange("e (hi tl) f -> (e hi) (tl f)", hi=_HI)
    ov = out[:].rearrange("(h c) (q gtl) -> q h c gtl", h=2, q=_NCHUNK)
    ovw = out[:].rearrange("(h c) (q g w tl) -> q w h c g tl", h=2,
                           q=_NCHUNK, g=_HI, w=2)

    with TileContext(nc) as tc:
        with (
            tc.tile_pool(name="const", bufs=1) as constp,
            tc.tile_pool(name="xin", bufs=8) as xin,
            tc.tile_pool(name="work", bufs=2) as work,
            tc.tile_pool(name="mm", bufs=2) as mmp,
            tc.tile_pool(name="psum", bufs=1, space="PSUM") as psp,
            tc.tile_pool(name="outp", bufs=4) as outp,
        ):
            # host-folded, pre-masked tables: [p, g*512 + (0:256 | 256:512)]
            # = (-8*scale*Ef.T@eq | alpha*(1-Ef).T@eq)[e] iff p == 4e+g else 0
            g2_sb = constp.tile([128, _HI * 512], BF16)
            nc.scalar.dma_start(out=g2_sb[:], in_=g2[:])

            def process(xt, col0, nt, q, w):
                """tree + select + matmuls + evac + out-DMA for nt t_lo cols
                of xt starting at column col0 (chunk q, half w for nt=256)."""
                x3 = xt[0:_P, col0 * _F:(col0 + nt) * _F].rearrange(
                    "p (tl f) -> p tl f", f=_F)
                # all-bf16 3-level add tree: every op has 2-byte src+dst with
                # unit inner stride, so DVE runs in its 2x perf mode
                l1 = work.tile([128, nt * 4], BF16, name="l1")
                nc.vector.tensor_tensor(out=l1[0:_P, 0:nt * 4],
                                        in0=x3[:, :, 0:4], in1=x3[:, :, 4:8],
                                        op=ALU.add)
                l13 = l1[0:_P, 0:nt * 4].rearrange("p (tl f) -> p tl f", f=4)
                l2 = work.tile([128, nt * 2], BF16, name="l2")
                nc.vector.tensor_tensor(out=l2[0:_P, 0:nt * 2],
                                        in0=l13[:, :, 0:2], in1=l13[:, :, 2:4],
                                        op=ALU.add)
                l23 = l2[0:_P, 0:nt * 2].rearrange("p (tl f) -> p tl f", f=2)
                s_raw = work.tile([128, nt], BF16, name="s_raw")
                nc.vector.tensor_tensor(out=s_raw[0:_P, 0:nt],
                                        in0=l23[:, :, 0:1], in1=l23[:, :, 1:2],
                                        op=ALU.add)

                obsf = mmp.tile([128, nt], BF16, name="obsf")
                nc.vector.tensor_tensor(out=obsf[0:_P, 0:nt],
                                        in0=s_raw[0:_P, 0:nt],
                                        in1=s_raw[0:_P, 0:nt], op=ALU.is_equal)
                s0 = mmp.tile([128, nt], BF16, name="s0")
                nc.vector.scalar_tensor_tensor(out=s0[0:_P, 0:nt],
                                               in0=s_raw[0:_P, 0:nt],
                                               scalar=3.0e38,
                                               in1=obsf[0:_P, 0:nt],
                                               op0=ALU.min, op1=ALU.mult)

                # out[c, t] matmuls: lhsT = constant table c-half, rhs = the
                # whole chunk of s0/obsf (N=nt, contiguous) -> 16 matmuls of
                # N=512 per chunk instead of 32 of N=256.  Each (h, g) block
                # gets its own og tile + 128KB write DMA issued right after
                # its evac, keeping the write stream smooth so it overlaps
                # the read stream instead of bunching at the end.
                for h in range(2):
                    for g in range(_HI):
                        pt = psp.tile([128, nt], F32, name=f"pt{h}{g}")
                        nc.tensor.matmul(pt[:, 0:nt],
                                         g2_sb[0:_P, g * 512 + h * 128:
                                               g * 512 + h * 128 + 128],
                                         s0[0:_P, 0:nt],
                                         start=True, stop=False)
                        nc.tensor.matmul(pt[:, 0:nt],
                                         g2_sb[0:_P, g * 512 + _C + h * 128:
                                               g * 512 + _C + h * 128 + 128],
                                         obsf[0:_P, 0:nt],
                                         start=False, stop=True)
                        og = outp.tile([128, nt], BF16, name=f"og{h}{g}")
                        if g == 3:
                            nc.vector.tensor_copy(out=og[:, 0:nt],
                                                  in_=pt[:, 0:nt])
                        else:
                            nc.scalar.copy(out=og[:, 0:nt], in_=pt[:, 0:nt])
                        if nt == _NT:
                            dst = ov[q, h][:, g * nt:(g + 1) * nt]
                        else:
                            dst = ovw[q, w, h][:, g, :]
                        nc.sync.dma_start(out=dst, in_=og[:, 0:nt])

            for ci in range(_NCHUNK):
                # SWDGE cast-DMA: f32 in HBM -> bf16 in SBUF (halves the DVE
                # stream volume; rides the Pool queue, separate from the
                # HWDGE write ring so read/write streams overlap)
                xt = xin.tile([128, _NT * _F], BF16, name="xt")
                with tc.high_priority():
                    nc.gpsimd.dma_start(out=xt[0:_P, :],
                                        in_=xv[:, ci * _NT * _F:(ci + 1) * _NT * _F])
                if ci == 0 or ci == _NCHUNK - 1:
                    # split the first chunk (earlier first write) and the
                    # final chunk (shorter pipeline drain)
                    process(xt, 0, 256, ci, 0)
                    process(xt, 256, 256, ci, 1)
                else:
                    process(xt, 0, _NT, ci, 0)
    nc.compile()
    return nc


def _get_nc():
    if "nc" not in _cache:
        _ensure_axon_hooks()
        from concourse import bass_utils
        bass_utils.upload_artifacts = lambda tmpdir: "local://skipped"
        _cache["nc"] = _build()
    return _cache["nc"]


def kernel(input_seq, eq_classes, scale, alpha, edge_present):
    global LAST_RESULTS
    x = np.asarray(input_seq, dtype=np.float32)
    eqc = np.asarray(eq_classes, dtype=np.float32)
    ef = np.asarray(edge_present).astype(np.float32)
    sc = float(np.asarray(scale))
    al = float(np.asarray(alpha))

    # host-side fold of the assembly axis (tiny matmul), pre-masked per t_hi
    # group; the GpSimd avg-pool divides s by 8, folded into G_edge here.
    import ml_dtypes
    g_edge = (ef.T @ eqc) * (-sc)              # (E, C)
    g_no = ((1.0 - ef).T @ eqc) * al           # (E, C)
    g2 = np.zeros((128, _HI * 512), np.float32)
    for e in range(_E):
        for g in range(_HI):
            p = _HI * e + g
            g2[p, g * 512:g * 512 + _C] = g_edge[e]
            g2[p, g * 512 + _C:(g + 1) * 512] = g_no[e]
    g2 = g2.astype(ml_dtypes.bfloat16)

    nc = _get_nc()
    from concourse import bass_utils
    in_maps = [{"x": np.ascontiguousarray(x[b]), "g2": g2}
               for b in range(_B)]
    trace = bool(os.environ.get("KERNEL_TRACE"))
    res = bass_utils.run_bass_kernel_spmd(nc, in_maps, core_ids=list(range(_B)),
                                          trace=trace)
    LAST_RESULTS = res
    outs = []
    for b in range(_B):
        # stored [c, q, g, tl]; logical t = g*4096 + q*512 + tl
        a = np.asarray(res.results[b]["out"]).reshape(_C, _NCHUNK, _HI, _NT)
        outs.append(a.transpose(2, 1, 3, 0).reshape(_T, _C).astype(np.float32))
    return np.stack(outs, axis=0)


# revision 23
# speedup vs baseline: 1.2886x; 1.2886x over previous
"""AssemblyClassifier Trainium2 kernel: 8-way batch-parallel across NeuronCores.

Per core (batch b), x = input_seq[b] : (28, 16384, 8) f32 is viewed flat as
[112 partitions = 4*e + t_hi, (t_lo, f)] (a contiguous split, so one full-rate
128-partition DMA per 512-t_lo chunk).  The assembly fold
G2 = [-8*scale*Ef | alpha*(1-Ef)].T @ eq_classes is computed on HOST (tiny
matmul) and shipped pre-masked per t_hi group as a [128, 4*512] bf16 table, so
the device preamble is one 512KB DMA.  Per chunk: the feature sum runs as one
DVE tensor_reduce over the innermost f=8 axis; GpSimd (otherwise idle)
computes the NaN mask (is_equal); DVE does the fused min/mult select to bf16;
32 K=112 matmuls per chunk use the 4 pre-masked tables with matmul columns
t-interleaved (j::4) so each out-DMA partition writes contiguous 2KB runs;
PSUM evac runs on ACT.  Read DMAs ride the
SP HWDGE ring (nc.sync) and output writes ride the ACT ring (nc.scalar), so
the two streams round-robin on the 16 SDMA engines and overlap instead of
serializing (the previous version put both on one ring: 82us DMA span for
23MB).  Output is written bf16 and upcast to f32 on host.
"""
import os
import sys
import types

import numpy as np

_B, _E, _T, _F = 8, 28, 16384, 8
_A, _C = 1024, 256
_HI = 4                 # t_hi groups (partition dim = 4*e + t_hi)
_TL = _T // _HI         # 4096 t_lo per group
_NT = 512               # t_lo chunk
_NJ = _NT // 128        # 128-col groups per chunk
_NCHUNK = _TL // _NT    # 8
_P = _E * _HI           # 112 active partitions

_cache = {}
LAST_RESULTS = None


def _ensure_axon_hooks():
    """The RL image's antenv lacks axon_hooks; shim it so trace=True works."""
    try:
        import antenv.axon_hooks  # noqa: F401
        return
    except Exception:
        pass
    try:
        from trn_agent_boot.trn_boot import _ntff_profile_via_ctypes
        hook = _ntff_profile_via_ctypes('/opt/axon/libaxon_pjrt.so')
    except Exception:
        hook = None
    m = types.ModuleType('antenv.axon_hooks')
    m.get_axon_ntff_profile_hook = lambda: hook
    m.set_axon_ntff_profile_hook = lambda h: None
    sys.modules['antenv.axon_hooks'] = m


def _build():
    import concourse.bass as bass
    import concourse.mybir as mybir
    from concourse import bacc
    from concourse.tile import TileContext

    F32 = mybir.dt.float32
    BF16 = mybir.dt.bfloat16
    ALU = mybir.AluOpType

    nc = bacc.Bacc("TRN2", target_bir_lowering=False)
    x = nc.declare_dram_parameter("x", [_E, _T, _F], F32, isOutput=False)
    g2 = nc.declare_dram_parameter("g2", [128, _HI * 512], BF16, isOutput=False)
    # transposed output: out[c, q*2048 + g*512 + tl]; host un-transposes.
    # Each chunk writes two [128c, 4KB-contiguous] DMAs - max-efficiency runs.
    out = nc.declare_dram_parameter("out", [_C, _T], BF16, isOutput=True)

    # flat [112, (t_lo f)] view of x; partition p = 4*e + t_hi
    xv = x[:].rearrange("e (hi tl) f -> (e hi) (tl f)", hi=_HI)
    ov = out[:].rearrange("(h c) (q gtl) -> q h c gtl", h=2, q=_NCHUNK)
    ovw = out[:].rearrange("(h c) (q g w tl) -> q w h c g tl", h=2,
                           q=_NCHUNK, g=_HI, w=2)

    with TileContext(nc) as tc:
        with (
            tc.tile_pool(name="const", bufs=1) as constp,
            tc.tile_pool(name="xin", bufs=8) as xin,
            tc.tile_pool(name="work", bufs=2) as work,
            tc.tile_pool(name="mm", bufs=2) as mmp,
            tc.tile_pool(name="psum", bufs=1, space="PSUM") as psp,
            tc.tile_pool(name="outp", bufs=4) as outp,
        ):
            # host-folded, pre-masked tables: [p, g*512 + (0:256 | 256:512)]
            # = (-8*scale*Ef.T@eq | alpha*(1-Ef).T@eq)[e] iff p == 4e+g else 0
            g2_sb = constp.tile([128, _HI * 512], BF16)
            nc.scalar.dma_start(out=g2_sb[:], in_=g2[:])

            def process(xt, col0, nt, q, w):
                """tree + select + matmuls + evac + out-DMA for nt t_lo cols
                of xt starting at column col0 (chunk q, half w for nt=256)."""
                x3 = xt[0:_P, col0 * _F:(col0 + nt) * _F].rearrange(
                    "p (tl f) -> p tl f", f=_F)
                # all-bf16 3-level add tree: every op has 2-byte src+dst with
                # unit inner stride, so DVE runs in its 2x perf mode
                l1 = work.tile([128, nt * 4], BF16, name="l1")
                nc.vector.tensor_tensor(out=l1[0:_P, 0:nt * 4],
                                        in0=x3[:, :, 0:4], in1=x3[:, :, 4:8],
                                        op=ALU.add)
                l13 = l1[0:_P, 0:nt * 4].rearrange("p (tl f) -> p tl f", f=4)
                l2 = work.tile([128, nt * 2], BF16, name="l2")
                nc.vector.tensor_tensor(out=l2[0:_P, 0:nt * 2],
                                        in0=l13[:, :, 0:2], in1=l13[:, :, 2:4],
                                        op=ALU.add)
                l23 = l2[0:_P, 0:nt * 2].rearrange("p (tl f) -> p tl f", f=2)
                s_raw = work.tile([128, nt], BF16, name="s_raw")
                nc.vector.tensor_tensor(out=s_raw[0:_P, 0:nt],
                                        in0=l23[:, :, 0:1], in1=l23[:, :, 1:2],
                                        op=ALU.add)

                obsf = mmp.tile([128, nt], BF16, name="obsf")
                nc.vector.tensor_tensor(out=obsf[0:_P, 0:nt],
                                        in0=s_raw[0:_P, 0:nt],
                                        in1=s_raw[0:_P, 0:nt], op=ALU.is_equal)
                s0 = mmp.tile([128, nt], BF16, name="s0")
                nc.vector.scalar_tensor_tensor(out=s0[0:_P, 0:nt],
                                               in0=s_raw[0:_P, 0:nt],
                                               scalar=3.0e38,
                                               in1=obsf[0:_P, 0:nt],
                                               op0=ALU.min, op1=ALU.mult)

                # out[c, t] matmuls: lhsT = constant table c-half, rhs = the
                # whole chunk of s0/obsf (N=nt, contiguous) -> 16 matmuls of
                # N=512 per chunk instead of 32 of N=256.  Each c-half's
                # 512KB write issues as soon as its 4 evacs are done.
                for h in range(2):
                    og = outp.tile([128, _HI * nt], BF16, name=f"og{h}")
                    for g in range(_HI):
                        pt = psp.tile([128, nt], F32, name=f"pt{h}{g}")
                        nc.tensor.matmul(pt[:, 0:nt],
                                         g2_sb[0:_P, g * 512 + h * 128:
                                               g * 512 + h * 128 + 128],
                                         s0[0:_P, 0:nt],
                                         start=True, stop=False)
                        nc.tensor.matmul(pt[:, 0:nt],
                                         g2_sb[0:_P, g * 512 + _C + h * 128:
                                               g * 512 + _C + h * 128 + 128],
                                         obsf[0:_P, 0:nt],
                                         start=False, stop=True)
                        dst = og[:, g * nt:(g + 1) * nt]
                        if g == 3:
                            nc.vector.tensor_copy(out=dst, in_=pt[:, 0:nt])
                        else:
                            nc.scalar.copy(out=dst, in_=pt[:, 0:nt])
                    if nt == _NT:
                        nc.sync.dma_start(out=ov[q, h], in_=og[:, :])
                    else:
                        nc.sync.dma_start(out=ovw[q, w, h],
                                          in_=og[:, :].rearrange(
                                              "c (g tl) -> c g tl", g=_HI))

            for ci in range(_NCHUNK):
                # SWDGE cast-DMA: f32 in HBM -> bf16 in SBUF (halves the DVE
                # stream volume; rides the Pool queue, separate from the
                # HWDGE write ring so read/write streams overlap)
                xt = xin.tile([128, _NT * _F], BF16, name="xt")
                with tc.high_priority():
                    nc.gpsimd.dma_start(out=xt[0:_P, :],
                                        in_=xv[:, ci * _NT * _F:(ci + 1) * _NT * _F])
                if ci == 0 or ci == _NCHUNK - 1:
                    # split the first chunk (earlier first write) and the
                    # final chunk (shorter pipeline drain)
                    process(xt, 0, 256, ci, 0)
                    process(xt, 256, 256, ci, 1)
                else:
                    process(xt, 0, _NT, ci, 0)
    nc.compile()
    return nc


def _get_nc():
    if "nc" not in _cache:
        _ensure_axon_hooks()
        from concourse import bass_utils
        bass_utils.upload_artifacts = lambda tmpdir: "local://skipped"
        _cache["nc"] = _build()
    return _cache["nc"]


def kernel(input_seq, eq_classes, scale, alpha, edge_present):
    global LAST_RESULTS
    x = np.asarray(input_seq, dtype=np.float32)
    eqc = np.asarray(eq_classes, dtype=np.float32)
    ef = np.asarray(edge_present).astype(np.float32)
    sc = float(np.asarray(scale))
    al = float(np.asarray(alpha))

    # host-side fold of the assembly axis (tiny matmul), pre-masked per t_hi
    # group; the GpSimd avg-pool divides s by 8, folded into G_edge here.
    import ml_dtypes
    g_edge = (ef.T @ eqc) * (-sc)              # (E, C)
    g_no = ((1.0 - ef).T @ eqc) * al           # (E, C)
    g2 = np.zeros((128, _HI * 512), np.float32)
    for e in range(_E):
        for g in range(_HI):
            p = _HI * e + g
            g2[p, g * 512:g * 512 + _C] = g_edge[e]
            g2[p, g * 512 + _C:(g + 1) * 512] = g_no[e]
    g2 = g2.astype(ml_dtypes.bfloat16)

    nc = _get_nc()
    from concourse import bass_utils
    in_maps = [{"x": np.ascontiguousarray(x[b]), "g2": g2}
               for b in range(_B)]
    trace = bool(os.environ.get("KERNEL_TRACE"))
    res = bass_utils.run_bass_kernel_spmd(nc, in_maps, core_ids=list(range(_B)),
                                          trace=trace)
    LAST_RESULTS = res
    outs = []
    for b in range(_B):
        # stored [c, q, g, tl]; logical t = g*4096 + q*512 + tl
        a = np.asarray(res.results[b]["out"]).reshape(_C, _NCHUNK, _HI, _NT)
        outs.append(a.transpose(2, 1, 3, 0).reshape(_T, _C).astype(np.float32))
    return np.stack(outs, axis=0)
